# revision 2
# baseline (speedup 1.0000x reference)
"""CARAFE upsampling kernel for 8 Trainium2 NeuronCores.

Problem (hardcoded): features (2,256,128,128) f32, masks (2,25,256,256) f32,
out (2,256,256,256) f32.  K=5, G=1, scale=2.

Strategy
--------
Sharding: 8 cores = batch(2) x H-half(2) x W-half(2).  Each core owns the
full C=256 and a 64x64 source patch (128x128 output patch), with a 2-pixel
feature halo on each side (halo'd on host, zero-padded at image edges).

Compute: for each source row pair si and channel half ch, the 25-tap
dynamic-filter sum is expressed as 5 PSUM-accumulated TensorE matmuls
(one per dy):

    out[c, (a,j)] = sum_dy  featT[x, y=si+dy, c]^T  @  band[si][x, dy, (a,j)]

where the "band" matrix holds mask values placed along x = j//2 + dx
diagonals (built on host, zero elsewhere).  Contraction K = 68 source
columns (64 + 4 halo), M = 128 channels, N = 256 output pixels (2 rows x
128 cols).  Output is produced directly in (c, i, j) layout.

featT and bands are shipped as fp16 (halves DMA traffic, enables FWL
weight loads); PSUM accumulation is fp32.
"""

import numpy as np
import concourse.bacc as bacc
import concourse.bass as bass
import concourse.mybir as mybir
import concourse.tile as tile
from concourse.bass_utils import run_bass_kernel_spmd

FP16 = mybir.dt.float16
F32 = mybir.dt.float32

N_CORES = 8
C = 256
H = W = 128
SI = 64          # source rows per core
SX = 64          # source cols per core
YR = SI + 4      # feature rows incl. halo
XR = SX + 4      # feature cols incl. halo (= matmul contraction K)
OI = 2 * SI      # output rows per core
OJ = 2 * SX      # output cols per core

_CACHED_NC = None
TRACE = False
_LAST_RESULTS = None  # BassKernelResults of the most recent run (for profiling)


def _build_nc():
    nc = bacc.Bacc(None, target_bir_lowering=False, debug=False)

    featT_d = nc.dram_tensor("featT", [XR, YR, C], FP16, kind="ExternalInput")
    bands_d = nc.dram_tensor("bands", [SI, XR, 5 * 2 * OJ], FP16, kind="ExternalInput")
    out_d = nc.dram_tensor("out", [C, OI, OJ], F32, kind="ExternalOutput")

    with tile.TileContext(nc) as tc:
        with (
            tc.tile_pool(name="feat", bufs=1) as fpool,
            tc.tile_pool(name="bands", bufs=4) as bpool,
            tc.tile_pool(name="psum", bufs=4, space=bass.MemorySpace.PSUM) as ppool,
            tc.tile_pool(name="outs", bufs=4) as opool,
        ):
            featT = fpool.tile([XR, YR * C], FP16)
            nc.sync.dma_start(featT[:], featT_d.rearrange("x y c -> x (y c)"))

            for si in range(SI):
                btile = bpool.tile([XR, 5 * 2 * OJ], FP16)
                nc.sync.dma_start(btile[:], bands_d[si])
                for ch in range(2):
                    ps = ppool.tile([128, 2 * OJ], F32)
                    for dy in range(5):
                        y = si + dy
                        off = y * C + ch * 128
                        nc.tensor.matmul(
                            ps[:],
                            featT[:, off : off + 128],
                            btile[:, dy * 2 * OJ : (dy + 1) * 2 * OJ],
                            start=(dy == 0),
                            stop=(dy == 4),
                        )
                    ot = opool.tile([128, 2 * OJ], F32)
                    if ch == 0:
                        nc.vector.tensor_copy(ot[:], ps[:])
                    else:
                        nc.scalar.copy(ot[:], ps[:])
                    dst = out_d[ch * 128 : (ch + 1) * 128, 2 * si : 2 * si + 2, :]
                    nc.sync.dma_start(dst.rearrange("c a j -> c (a j)"), ot[:])

    nc.compile()
    return nc


def _get_nc():
    global _CACHED_NC
    if _CACHED_NC is None:
        _CACHED_NC = _build_nc()
    return _CACHED_NC


def _prep_core_inputs(features: np.ndarray, masks: np.ndarray):
    """Build per-core featT + band tensors (host-side sharding)."""
    fp = np.pad(features, ((0, 0), (0, 0), (2, 2), (2, 2)))  # (2,256,132,132)

    # band scatter indices (shared by all cores)
    sj = np.arange(SX)
    in_maps = []
    for core in range(N_CORES):
        n, hb, wb = core // 4, (core // 2) % 2, core % 2

        fsl = fp[n, :, hb * SI : hb * SI + YR, wb * SX : wb * SX + XR]
        featT = np.ascontiguousarray(fsl.transpose(2, 1, 0)).astype(
            np.float16
        )  # (x, y, c)

        msl = masks[n, :, hb * OI : (hb + 1) * OI, wb * OJ : (wb + 1) * OJ]
        # m6[dy, dx, si, a, sjj, b]
        m6 = msl.reshape(5, 5, SI, 2, SX, 2)
        bands = np.zeros((SI, XR, 5, 2, SX, 2), dtype=np.float32)
        for dx in range(5):
            # advanced indices (axis1: sj+dx, axis4: sj) broadcast together and
            # land in front: target shape [sj, si, dy, a, b]
            bands[:, sj + dx, :, :, sj, :] = m6[:, dx].transpose(3, 1, 0, 2, 4)
        bands = bands.astype(np.float16).reshape(SI, XR, 5 * 2 * OJ)

        in_maps.append({"featT": featT, "bands": bands})
    return in_maps


def kernel(features: np.ndarray, masks: np.ndarray) -> np.ndarray:
    global _LAST_RESULTS
    features = np.asarray(features, dtype=np.float32)
    masks = np.asarray(masks, dtype=np.float32)

    nc = _get_nc()
    in_maps = _prep_core_inputs(features, masks)
    res = run_bass_kernel_spmd(nc, in_maps, list(range(N_CORES)), trace=TRACE)
    _LAST_RESULTS = res

    out = np.empty((2, C, 256, 256), dtype=np.float32)
    for core in range(N_CORES):
        n, hb, wb = core // 4, (core // 2) % 2, core % 2
        out[n, :, hb * OI : (hb + 1) * OI, wb * OJ : (wb + 1) * OJ] = res.results[
            core
        ]["out"]
    return out


# revision 19
# speedup vs baseline: 1.8297x; 1.8297x over previous
"""CARAFE upsampling kernel for 8 Trainium2 NeuronCores.

Problem (hardcoded): features (2,256,128,128) f32, masks (2,25,256,256) f32,
out (2,256,256,256) f32.  K=5, G=1, scale=2.

Strategy
--------
Sharding: 8 cores = batch(2) x H-half(2) x W-half(2).  Each core owns the
full C=256 and a 64x64 source patch (128x128 output patch), with a 2-pixel
feature halo on each side (halo'd on host, zero-padded at image edges).

Compute: the 25-tap dynamic-filter sum becomes PSUM-accumulated TensorE
matmuls.  For source-row pair si, channel half ch, tap row dy, and output
column half h:

    out[c, (a, jj)] += featT[x, y=si+dy, c]^T  @  band[x, (a, jj)]

where the band matrix holds mask values on x = jj//2 + dx diagonals (built
host-side; zeros elsewhere).  The j-range is split in half (h) so the
contraction is only K=36 (instead of 68), which nearly halves the band
bytes shipped from HBM.  The h=1 window (x in [32,68)) is placed at
partition base 64 (tile_position row alignment), fed by a duplicated
feature block.  M=128 channels, N=128 output pixels per matmul; fp16
operands, fp32 PSUM accumulate; output leaves in native (c, i, j) layout.

DMAs are batched in groups of 8 si (two band loads + two staged output
stores per group) to amortize HWDGE/sequencer fixed costs.
"""

import numpy as np

import concourse.bacc as bacc
import concourse.bass as bass
import concourse.mybir as mybir
import concourse.tile as tile
from concourse.bass_utils import run_bass_kernel_spmd

FP16 = mybir.dt.float16
F32 = mybir.dt.float32

N_CORES = 8
C = 256
SI = 64            # source rows per core
SX = 64            # source cols per core
YR = SI + 4        # feature rows incl. halo
XW = 36            # contraction window per column half (32 + 4 halo)
OI = 2 * SI        # output rows per core
OJ = 2 * SX        # output cols per core
GROUP = 8          # si per DMA/staging group
NG = SI // GROUP
BSI = 5 * 2 * 64   # band elems per (si, h): 5 dy x (2a x 64 jj)

_CACHED_NC = None
TRACE = False
_LAST_RESULTS = None  # BassKernelResults of the most recent run


def _build_nc():
    nc = bacc.Bacc(None, target_bir_lowering=False, debug=False)

    # rows 0:36 -> x in [0,36) (partitions 0:36); rows 36:72 -> x in [32,68)
    # (partitions 64:100, the 64-aligned duplicate block for h=1)
    featT_d = nc.dram_tensor("featT", [72, YR, C], FP16, kind="ExternalInput")
    # [h, x', si, dy, (a jj)]
    bands_d = nc.dram_tensor("bands", [2, XW, SI, 5, 128], FP16, kind="ExternalInput")
    out_d = nc.dram_tensor("out", [C, OI, OJ], F32, kind="ExternalOutput")

    with tile.TileContext(nc) as tc:
        with (
            tc.tile_pool(name="feat", bufs=1) as fpool,
            tc.tile_pool(name="bands", bufs=6) as bpool,
            tc.tile_pool(name="psum", bufs=8, space=bass.MemorySpace.PSUM) as ppool,
            tc.tile_pool(name="stage", bufs=6) as spool,
        ):
            # features arrive in y-chunks interleaved with the band groups so
            # group 0's matmuls can start after ~2 small DMAs
            ftile = fpool.tile([100, YR * C], FP16)
            fdone = [0, 0]

            def load_feat_rows(upto, h):
                nonlocal fdone
                if upto <= fdone[h]:
                    return
                nc.sync.dma_start(
                    ftile[64 * h : 64 * h + XW, fdone[h] * C : upto * C],
                    featT_d[36 * h : 36 * h + XW, fdone[h] : upto].rearrange(
                        "x y c -> x (y c)"
                    ),
                )
                fdone[h] = upto

            for g in range(NG):
                # h=0 lives on partitions 0:36, h=1 on 64:100 -- the two
                # halves share the same free range (disjoint partitions)
                btile = bpool.tile([100, GROUP * 640], FP16)
                for h in range(2):
                    src = bands_d[h, :, g * GROUP : (g + 1) * GROUP]
                    nc.sync.dma_start(
                        btile[64 * h : 64 * h + XW, :],
                        src.rearrange("x s d j -> x (s d j)"),
                    )
                    load_feat_rows(min(GROUP * (g + 3) + 4, YR), h)
                for ch in range(2):
                    stg = spool.tile([128, GROUP * 256], F32)
                    for sl in range(GROUP):
                        si = g * GROUP + sl
                        # psum columns are h-major (contiguous per matmul);
                        # the copy below unscrambles to (a, j) order
                        ps = ppool.tile([128, 256], F32)
                        for dy in range(5):
                            yoff = (si + dy) * C + ch * 128
                            for h in range(2):
                                boff = sl * 640 + dy * 128
                                # start once per bank: it clears has_written
                                # for the WHOLE bank, so h=1's first matmul
                                # must not re-clear (it would wipe h=0's dy0
                                # accumulation state); its fresh columns
                                # overwrite anyway since their bits are clear
                                nc.tensor.matmul(
                                    ps[:, 128 * h : 128 * h + 128],
                                    ftile[64 * h : 64 * h + XW, yoff : yoff + 128],
                                    btile[64 * h : 64 * h + XW, boff : boff + 128],
                                    start=(dy == 0 and h == 0),
                                    stop=(dy == 4 and h == 1),
                                    skip_group_check=True,
                                )
                        # stg col = a*128 + h*64 + jj, iterated in the psum's
                        # (h, a, jj) source order
                        dst = stg[:, sl * 256 : (sl + 1) * 256].rearrange(
                            "p (a h j) -> p h a j", a=2, h=2
                        )
                        if ch == 0:
                            nc.vector.tensor_copy(dst, ps[:])
                        else:
                            nc.scalar.copy(dst, ps[:])
                    odst = out_d[
                        ch * 128 : (ch + 1) * 128,
                        g * 2 * GROUP : (g + 1) * 2 * GROUP,
                        :,
                    ]
                    nc.scalar.dma_start(odst.rearrange("c a j -> c (a j)"), stg[:])

    nc.compile()
    return nc


def _get_nc():
    global _CACHED_NC
    if _CACHED_NC is None:
        _CACHED_NC = _build_nc()
    return _CACHED_NC


def _prep_core_inputs(features: np.ndarray, masks: np.ndarray):
    """Build per-core featT + band tensors (host-side sharding)."""
    fp = np.pad(features, ((0, 0), (0, 0), (2, 2), (2, 2)))  # (2,256,132,132)

    sjl = np.arange(32)
    in_maps = []
    for core in range(N_CORES):
        n, hb, wb = core // 4, (core // 2) % 2, core % 2

        fsl = fp[n, :, hb * SI : hb * SI + YR, wb * SX : wb * SX + SX + 4]
        featT = np.ascontiguousarray(fsl.transpose(2, 1, 0))  # (x, y, c) fp32
        featT = np.concatenate([featT[0:36], featT[32:68]], axis=0).astype(np.float16)

        msl = masks[n, :, hb * OI : (hb + 1) * OI, wb * OJ : (wb + 1) * OJ]
        # m6[dy, dx, si, a, sj, b]
        m6 = msl.reshape(5, 5, SI, 2, SX, 2)
        # bh[h, x', si, dy, a, sjl, b]
        bh = np.zeros((2, XW, SI, 5, 2, 32, 2), dtype=np.float32)
        for h in range(2):
            for dx in range(5):
                # m6[:, dx] -> [dy, si, a, sj, b] -> [sj, si, dy, a, b],
                # sliced to this column half's sj range
                mh = m6[:, dx].transpose(3, 1, 0, 2, 4)[32 * h : 32 * h + 32]
                # advanced axes (x' at 0, sjl at 4) land in front:
                # target [sjl, si, dy, a, b]
                bh[h, sjl + dx, :, :, :, sjl, :] = mh
        bands = bh.astype(np.float16).reshape(2, XW, SI, 5, 128)

        in_maps.append({"featT": featT, "bands": bands})
    return in_maps


def kernel(features: np.ndarray, masks: np.ndarray) -> np.ndarray:
    global _LAST_RESULTS
    features = np.asarray(features, dtype=np.float32)
    masks = np.asarray(masks, dtype=np.float32)

    nc = _get_nc()
    in_maps = _prep_core_inputs(features, masks)
    res = run_bass_kernel_spmd(nc, in_maps, list(range(N_CORES)), trace=TRACE)
    _LAST_RESULTS = res

    out = np.empty((2, C, 256, 256), dtype=np.float32)
    for core in range(N_CORES):
        n, hb, wb = core // 4, (core // 2) % 2, core % 2
        out[n, :, hb * OI : (hb + 1) * OI, wb * OJ : (wb + 1) * OJ] = res.results[
            core
        ]["out"]
    return out


# revision 20
# speedup vs baseline: 1.8670x; 1.0203x over previous
"""CARAFE upsampling kernel for 8 Trainium2 NeuronCores.

Problem (hardcoded): features (2,256,128,128) f32, masks (2,25,256,256) f32,
out (2,256,256,256) f32.  K=5, G=1, scale=2 (CARAFE content-aware upsample).

Strategy
--------
Sharding: 8 cores = batch(2) x H-half(2) x W-half(2).  Each core owns the
full C=256 and a 64x64 source patch (128x128 output patch) with a 2-pixel
feature halo (sliced with halo / zero-padded on host).

Compute: the 25-tap dynamic-filter sum becomes PSUM-accumulated TensorE
matmuls.  For source-row pair si, channel half ch, tap row dy, and output
column half h:

    out[c, (a, jj)] += featT[x', y=si+dy, c]^T  @  band[x', (a, jj)]

The band matrix (built host-side) holds mask values along x = jj//2 + dx
diagonals, zeros elsewhere.  Splitting the j-range in half (h) keeps the
contraction at K=36 (32 + 4 halo) instead of 68, nearly halving the band
bytes shipped from HBM.  The h=1 window (x in [32,68)) is x-REVERSED on the
host so both halves contract at partition base 0 (the contraction sum is
order-invariant); partition-offset matmuls crash the runtime here.  Each
matmul writes a contiguous 128-col PSUM slice (h-major); a single
start=True per PSUM bank clears has_written for the whole bank, so h=1's
first matmul (start=False) lands on cleared bits and overwrites.  M=128
channels, N=128 pixels/matmul; fp16 operands (rel err ~3.5e-4), fp32 PSUM.

Output leaves in native (c, i, j) layout via a PSUM->SBUF copy that
unscrambles (h, a, jj) -> (a, j), staged 8 si at a time into 1 MiB stores.
DMAs are batched per 8-si group to amortize HWDGE/sequencer fixed costs;
features become fully SBUF-resident by group 2.

TimelineSim cost model: ~82 us/core; PE busy ~70 us, DMA ~70 us.
"""

import numpy as np

import concourse.bacc as bacc
import concourse.bass as bass
import concourse.mybir as mybir
import concourse.tile as tile
from concourse.bass_utils import run_bass_kernel_spmd

FP16 = mybir.dt.float16
F32 = mybir.dt.float32

N_CORES = 8
C = 256
SI = 64
SX = 64
YR = SI + 4
XW = 36
OI = 2 * SI
OJ = 2 * SX
GROUP = 8
NG = SI // GROUP

_CACHED_NC = None
TRACE = False
_LAST_RESULTS = None


def _build_nc():
    nc = bacc.Bacc(None, target_bir_lowering=False, debug=False)

    # [hblock, x', y, c]: block 0 = x 0:36; block 1 = x 67..32 (reversed)
    featT_d = nc.dram_tensor("featT", [2, XW, YR, C], FP16, kind="ExternalInput")
    # [h, x', si, dy, (a jj)]; h=1 x'-axis reversed to match featT block 1
    bands_d = nc.dram_tensor("bands", [2, XW, SI, 5, 128], FP16, kind="ExternalInput")
    out_d = nc.dram_tensor("out", [C, OI, OJ], F32, kind="ExternalOutput")

    with tile.TileContext(nc) as tc:
        with (
            tc.tile_pool(name="feat", bufs=1) as fpool,
            tc.tile_pool(name="bands", bufs=3) as bpool,
            tc.tile_pool(name="psum", bufs=8, space=bass.MemorySpace.PSUM) as ppool,
            tc.tile_pool(name="stage", bufs=4) as spool,
        ):
            ftiles = [
                fpool.tile([XW, YR * C], FP16, tag=f"ft{h}", name=f"ft{h}")
                for h in range(2)
            ]
            fdone = [0, 0]

            def load_feat_rows(upto, h):
                nonlocal fdone
                if upto <= fdone[h]:
                    return
                nc.sync.dma_start(
                    ftiles[h][:, fdone[h] * C : upto * C],
                    featT_d[h, :, fdone[h] : upto].rearrange("x y c -> x (y c)"),
                )
                fdone[h] = upto

            for g in range(NG):
                btiles = [
                    bpool.tile([XW, GROUP * 640], FP16, tag=f"bt{h}", name=f"bt{h}")
                    for h in range(2)
                ]
                for h in range(2):
                    # group 0: halve the first transfers so matmuls start early
                    splits = [(0, 4), (4, 8)] if g == 0 else [(0, GROUP)]
                    for s0, s1 in splits:
                        src = bands_d[h, :, g * GROUP + s0 : g * GROUP + s1]
                        nc.sync.dma_start(
                            btiles[h][:, s0 * 640 : s1 * 640],
                            src.rearrange("x s d j -> x (s d j)"),
                        )
                        if g == 0:
                            load_feat_rows(s1 + 4, h)
                    if g == 0:
                        load_feat_rows(GROUP * 2 + 4, h)
                    else:
                        load_feat_rows(YR, h)
                for ch in range(2):
                    stg = spool.tile([128, GROUP * 256], F32)
                    for sl in range(GROUP):
                        si = g * GROUP + sl
                        # psum columns are h-major (contiguous per matmul);
                        # the copy below unscrambles to (a, j) order
                        ps = ppool.tile([128, 256], F32)
                        for dy in range(5):
                            yoff = (si + dy) * C + ch * 128
                            for h in range(2):
                                boff = sl * 640 + dy * 128
                                # start once per bank (clears has_written for
                                # the WHOLE bank); h=1's first write lands on
                                # cleared bits and overwrites
                                nc.tensor.matmul(
                                    ps[:, 128 * h : 128 * h + 128],
                                    ftiles[h][:, yoff : yoff + 128],
                                    btiles[h][:, boff : boff + 128],
                                    start=(dy == 0 and h == 0),
                                    stop=(dy == 4 and h == 1),
                                    skip_group_check=True,
                                )
                        # stg col = a*128 + h*64 + jj, iterated in the psum's
                        # (h, a, jj) source order
                        dst = stg[:, sl * 256 : (sl + 1) * 256].rearrange(
                            "p (a h j) -> p h a j", a=2, h=2
                        )
                        if ch == 0:
                            nc.vector.tensor_copy(dst, ps[:])
                        else:
                            nc.scalar.copy(dst, ps[:])
                    odst = out_d[
                        ch * 128 : (ch + 1) * 128,
                        g * 2 * GROUP : (g + 1) * 2 * GROUP,
                        :,
                    ]
                    nc.scalar.dma_start(odst.rearrange("c a j -> c (a j)"), stg[:])

    nc.compile()
    return nc


def _get_nc():
    global _CACHED_NC
    if _CACHED_NC is None:
        _CACHED_NC = _build_nc()
    return _CACHED_NC


def _prep_core_inputs(features: np.ndarray, masks: np.ndarray):
    fp = np.pad(features, ((0, 0), (0, 0), (2, 2), (2, 2)))

    sjl = np.arange(32)
    in_maps = []
    for core in range(N_CORES):
        n, hb, wb = core // 4, (core // 2) % 2, core % 2

        fsl = fp[n, :, hb * SI : hb * SI + YR, wb * SX : wb * SX + SX + 4]
        featT = np.ascontiguousarray(fsl.transpose(2, 1, 0))  # (x, y, c)
        fA = featT[0:36]
        fB = featT[32:68][::-1]  # x reversed
        featT2 = np.stack([fA, fB]).astype(np.float16)

        msl = masks[n, :, hb * OI : (hb + 1) * OI, wb * OJ : (wb + 1) * OJ]
        m6 = msl.reshape(5, 5, SI, 2, SX, 2)
        bh = np.zeros((2, XW, SI, 5, 2, 32, 2), dtype=np.float32)
        for h in range(2):
            for dx in range(5):
                mh = m6[:, dx].transpose(3, 1, 0, 2, 4)[32 * h : 32 * h + 32]
                bh[h, sjl + dx, :, :, :, sjl, :] = mh
        bh[1] = bh[1][::-1]  # mirror x' to match featT block 1
        bands = bh.astype(np.float16).reshape(2, XW, SI, 5, 128)

        in_maps.append({"featT": featT2, "bands": bands})
    return in_maps


def kernel(features: np.ndarray, masks: np.ndarray) -> np.ndarray:
    global _LAST_RESULTS
    features = np.asarray(features, dtype=np.float32)
    masks = np.asarray(masks, dtype=np.float32)

    nc = _get_nc()
    in_maps = _prep_core_inputs(features, masks)
    res = run_bass_kernel_spmd(nc, in_maps, list(range(N_CORES)), trace=TRACE)
    _LAST_RESULTS = res

    out = np.empty((2, C, 256, 256), dtype=np.float32)
    for core in range(N_CORES):
        n, hb, wb = core // 4, (core // 2) % 2, core % 2
        out[n, :, hb * OI : (hb + 1) * OI, wb * OJ : (wb + 1) * OJ] = res.results[
            core
        ]["out"]
    return out
